# revision 12
# baseline (speedup 1.0000x reference)
"""Single-head attention (B=4, T=4096, C=1024, H=64) on 8 trn2 NeuronCores.

Sharding: 8 shards = (batch b, query-half h).  Each core receives x[b]
pre-transposed to xT [C=1024, T=4096]; for h==1 the T columns are rotated by
2048 so that "this core's" 2048 queries are always columns 0:2048 (softmax is
permutation-invariant over keys, so rotating the key order changes nothing).
This keeps the SPMD program identical on every core with no rank logic.

Per-core kernel (flash-attention style; the [T,T] score matrix never touches
DRAM):
  phase 1: stream xT in [128,512] tiles; PE computes KVT = [Wk|Wv]^T x^T
           ([128,4096], rows 0:64 = K^T, 64:128 = V^T) and Q^T [64,2048]
           (f32r matmuls, contraction over C in 8 chunks of 128);
           V^T tiles are PE-transposed back to V [s,64] and a ones column is
           appended (-> softmax denominator comes out of the attn@V matmul).
  phase 2: for each 512-wide query chunk: for each 128-key tile,
           PE: scoresT[s=128, t=512] = K_tile^T{64,128}.T @ Q^T{64,512}
           ACT: exp(0.125 * scoresT) -> SBUF   (scores are O(3), no max-sub
           needed for a numerically safe softmax)
           PE: outT[65,512] += V_aug[s,65].T @ exp  (accumulate over 32 tiles)
           then PE-transpose outT back to [t,65], multiply rows by the
           reciprocal of column 64 (the exp-sum), DMA out.
"""

import os
import sys

for _p in ("/opt/trn_rl_repo", "/root/.axon_site/_ro/trn_rl_repo"):
    if os.path.isdir(_p) and _p not in sys.path:
        sys.path.append(_p)

import numpy as np

import concourse.bacc as bacc
import concourse.mybir as mybir
import concourse.tile as tile
from concourse.bass_utils import run_bass_kernel_spmd
from concourse.masks import make_identity

B = 4
T = 4096
C = 1024
H = 64
TQ = T // 2  # queries per core
N_CORES = 8

F32 = mybir.dt.float32
F32R = mybir.dt.float32r

NC_CH = C // 128  # 8 contraction chunks
NSB = T // 512  # 8 key/source blocks of 512
NST = T // 128  # 32 key tiles of 128
NTC = TQ // 512  # 4 query chunks of 512


def _build_module():
    nc = bacc.Bacc("TRN2", target_bir_lowering=False, debug=False, num_devices=N_CORES)

    xT = nc.dram_tensor("xT", [C, T], F32, kind="ExternalInput").ap()
    wq = nc.dram_tensor("wq", [C, H], F32, kind="ExternalInput").ap()
    wkv = nc.dram_tensor("wkv", [C, 2 * H], F32, kind="ExternalInput").ap()
    out = nc.dram_tensor("out", [TQ, H], F32, kind="ExternalOutput").ap()

    EXP = mybir.ActivationFunctionType.Exp

    with tile.TileContext(nc) as tc:
        with (
            tc.tile_pool(name="const", bufs=1) as const_pool,
            tc.tile_pool(name="xt", bufs=24) as xt_pool,
            tc.tile_pool(name="big", bufs=1) as big_pool,
            tc.tile_pool(name="exp", bufs=4) as exp_pool,
            tc.tile_pool(name="outts", bufs=2) as outts_pool,
            tc.tile_pool(name="small", bufs=4) as small_pool,
            tc.tile_pool(name="pmm", bufs=2, space="PSUM") as psum_mm,
            tc.tile_pool(name="pacc", bufs=2, space="PSUM") as psum_acc,
            tc.tile_pool(name="pdum", bufs=1, space="PSUM") as psum_dum,
        ):
            # ---- constants ----
            wq_sb = const_pool.tile([128, NC_CH, H], F32R, tag="wq")
            wkv_sb = const_pool.tile([128, NC_CH, 2 * H], F32R, tag="wkv")
            ident_f32 = const_pool.tile([128, 128], F32, tag="ident_f32")
            ones_f32 = const_pool.tile([128, NST, 1], F32, tag="ones")
            for c in range(NC_CH):
                nc.sync.dma_start(wq_sb[:, c, :], wq[c * 128 : (c + 1) * 128, :].bitcast(F32R))
                nc.sync.dma_start(wkv_sb[:, c, :], wkv[c * 128 : (c + 1) * 128, :].bitcast(F32R))
            make_identity(nc, ident_f32[:])
            nc.gpsimd.memset(ones_f32[:], 1.0)
            # bf16 HAM-warmer operands: f32r matmuls appear not to register as
            # PE activity, so the clock gate re-throttles mid-kernel; a tiny
            # bf16 matmul every iteration keeps the PE at 2.4 GHz.
            BF16 = mybir.dt.bfloat16
            dwa = const_pool.tile([128, 128], BF16, tag="dwa")
            dwb = const_pool.tile([128, 64], BF16, tag="dwb")
            nc.gpsimd.memset(dwa[:], 0.0)
            nc.gpsimd.memset(dwb[:], 0.0)

            # ---- persistent activations ----
            kv_f32 = big_pool.tile([128, T], F32, tag="kvf32")  # K^T | V^T (f32)
            kt_sb = big_pool.tile([64, T], F32R, tag="kt")  # K^T rounded for PE
            qt_sb = big_pool.tile([64, TQ], F32R, tag="qt")  # Q^T
            va = big_pool.tile([128, NST, 66], F32R, tag="va")  # V_aug per s-tile
            nc.vector.tensor_copy(va[:, :, 64:65], ones_f32[:])

            # ---- phase 1: projections ----
            for sb in range(NSB):
                xts = []
                for c in range(NC_CH):
                    xt = xt_pool.tile([128, 512], F32R, tag="xt")
                    nc.sync.dma_start(
                        xt[:], xT[c * 128 : (c + 1) * 128, sb * 512 : (sb + 1) * 512].bitcast(F32R)
                    )
                    xts.append(xt)
                kvt_ps = psum_mm.tile([128, 512], F32, tag="mm")
                for c in range(NC_CH):
                    nc.tensor.matmul(
                        kvt_ps[:],
                        wkv_sb[:, c, :],
                        xts[c][:],
                        start=(c == 0),
                        stop=(c == NC_CH - 1),
                    )
                nc.vector.tensor_copy(kv_f32[:, sb * 512 : (sb + 1) * 512], kvt_ps[:])
                nc.vector.tensor_copy(kt_sb[:, sb * 512 : (sb + 1) * 512], kvt_ps[0:64, :])
                if sb < NTC:  # query half
                    qt_ps = psum_acc.tile([64, 512], F32, tag="acc")
                    for c in range(NC_CH):
                        nc.tensor.matmul(
                            qt_ps[:],
                            wq_sb[:, c, :],
                            xts[c][:],
                            start=(c == 0),
                            stop=(c == NC_CH - 1),
                        )
                    nc.vector.tensor_copy(qt_sb[:, sb * 512 : (sb + 1) * 512], qt_ps[:])
                for j in range(4):  # V tiles of this block
                    st = sb * 4 + j
                    vt_ps = psum_mm.tile([128, 64], F32, tag="mm")
                    nc.tensor.transpose(
                        vt_ps[:],
                        kv_f32[64:128, st * 128 : (st + 1) * 128],
                        ident_f32[64:128, 64:128],
                    )
                    nc.vector.tensor_copy(va[:, st, 0:64], vt_ps[:])

            # ---- phase 2: attention, two query chunks (1024 queries) at a time ----
            for tcp in range(NTC // 2):
                tc0 = 2 * tcp
                outt_a = psum_acc.tile([65, 512], F32, tag="acc")
                outt_b = psum_acc.tile([65, 512], F32, tag="acc")
                for st in range(NST):
                    dps = psum_dum.tile([128, 64], F32, tag="dum")
                    nc.tensor.matmul(dps[:], dwa[:], dwb[:], start=True, stop=True)
                    kt_slice = kt_sb[:, st * 128 : (st + 1) * 128]
                    sc_ps = psum_mm.tile([128, 1024], F32, tag="mm")
                    for i in range(2):
                        nc.tensor.matmul(
                            sc_ps[:, i * 512 : (i + 1) * 512],
                            kt_slice,
                            qt_sb[:, (tc0 + i) * 512 : (tc0 + i + 1) * 512],
                            start=True,
                            stop=True,
                        )
                    ex = exp_pool.tile([128, 1024], F32R, tag="exp")
                    nc.scalar.activation(ex[:], sc_ps[:], EXP, scale=0.125)
                    for i, outt_ps in enumerate((outt_a, outt_b)):
                        nc.tensor.matmul(
                            outt_ps[:],
                            va[:, st, 0:65],
                            ex[:, i * 512 : (i + 1) * 512],
                            start=(st == 0),
                            stop=(st == NST - 1),
                        )
                for i, outt_ps in enumerate((outt_a, outt_b)):
                    tci = tc0 + i
                    outt_sb = outts_pool.tile([65, 512], F32, tag="outts")
                    nc.vector.tensor_copy(outt_sb[:], outt_ps[:])
                    for k in range(4):
                        o_ps = psum_mm.tile([128, 65], F32, tag="mm")
                        nc.tensor.transpose(
                            o_ps[:], outt_sb[:, k * 128 : (k + 1) * 128], ident_f32[0:65, 0:65]
                        )
                        rc = small_pool.tile([128, 1], F32, tag="rc")
                        nc.vector.reciprocal(rc[:], o_ps[:, 64:65])
                        o_sb = small_pool.tile([128, H], F32, tag="osb")
                        nc.vector.tensor_scalar_mul(o_sb[:], o_ps[:, 0:H], rc[:])
                        row = tci * 512 + k * 128
                        nc.sync.dma_start(out[row : row + 128, :], o_sb[:])

    nc.compile()
    return nc


_NC_CACHE = None


def _get_module():
    global _NC_CACHE
    if _NC_CACHE is None:
        _NC_CACHE = _build_module()
    return _NC_CACHE


def _make_in_maps(x, Wq, Wk, Wv):
    xT = np.transpose(np.asarray(x, dtype=np.float32), (0, 2, 1))  # [B, C, T]
    wq = np.ascontiguousarray(np.asarray(Wq, dtype=np.float32))
    wkv = np.ascontiguousarray(
        np.concatenate(
            [np.asarray(Wk, dtype=np.float32), np.asarray(Wv, dtype=np.float32)],
            axis=1,
        )
    )
    in_maps = []
    for core in range(N_CORES):
        b, h = divmod(core, 2)
        xt = xT[b]
        if h == 1:
            xt = np.concatenate([xt[:, TQ:], xt[:, :TQ]], axis=1)
        in_maps.append(
            {"xT": np.ascontiguousarray(xt), "wq": wq, "wkv": wkv}
        )
    return in_maps


def run(x, Wq, Wk, Wv, **spmd_kwargs):
    """Run on hardware; returns (output, BassKernelResults)."""
    nc = _get_module()
    in_maps = _make_in_maps(x, Wq, Wk, Wv)
    res = run_bass_kernel_spmd(nc, in_maps, core_ids=list(range(N_CORES)), **spmd_kwargs)
    out = np.empty((B, T, H), dtype=np.float32)
    for core in range(N_CORES):
        b, h = divmod(core, 2)
        out[b, h * TQ : (h + 1) * TQ, :] = res.results[core]["out"]
    return out, res


def kernel(x, Wq, Wk, Wv):
    out, _ = run(x, Wq, Wk, Wv)
    return out
